# revision 6
# baseline (speedup 1.0000x reference)
"""Trainium2 Bass kernel for a cross-attention transformer block.

Reference computation (per batch element b of 8):
    kv = img_feat + img_pos                       [4096, 512]
    q = pattern @ Wq + bq                         [1024, 512]
    k = kv @ Wk + bk ; v = kv @ Wv + bv           [4096, 512]
    scores = q @ k.T                              [1024, 4096]
    attn = softmax(scores, axis=QUERY)            (normalized over the 1024 axis)
    x = attn @ v                                  [1024, 512]
    h = LN(pattern + x) ; out = LN(h + relu(h@W1+b1)@W2 + b2)

Sharding: pure data-parallel — batch 8 across 8 NeuronCores, one batch
element per core, no collectives.

Per-core layout strategy: activations kept feature-on-partition ("T"
layout) for all matmuls that contract over features. q/k/scores run as
float32r (full fp32 data, 1 cycle/row on the PE for moving dims >= 256);
attn/v/FFN run in bf16. Softmax over the query axis is a free-axis
reduce in the scores^T [kv_token_partition, query_free] layout, and the
softmax normalization is folded into v (v''[n] = v[n]/denom[n]) so the
attention matmul needs no extra normalization pass.
"""

import numpy as np

P = 128
C = 512          # feature dim
CK = C // P      # 4 feature chunks
M = 1024         # queries
MT = M // P      # 8 query tiles
N = 4096         # kv tokens
NT = 8           # n-tiles of 512
NC = N // P      # 32 kv chunks
D = 2048         # ffn hidden
DC = D // P      # 16 ffn chunks
B = 8            # batch == cores
EPS = 1e-5

_CACHE = {}


def _build():
    from contextlib import ExitStack

    import concourse.bacc as bacc
    import concourse.bass as bass
    import concourse.mybir as mybir
    import concourse.tile as tile
    from concourse.masks import make_identity

    f32 = mybir.dt.float32
    bf16 = mybir.dt.bfloat16
    f32r = mybir.dt.float32r
    Alu = mybir.AluOpType
    Act = mybir.ActivationFunctionType
    AX = mybir.AxisListType

    nc = bacc.Bacc("TRN2", target_bir_lowering=False, debug=False, num_devices=B)

    img_feat = nc.dram_tensor("img_feat", (N, C), f32, kind="ExternalInput")
    img_pos = nc.dram_tensor("img_pos", (N, C), f32, kind="ExternalInput")
    pattern = nc.dram_tensor("pattern_feat", (M, C), f32, kind="ExternalInput")
    Wq = nc.dram_tensor("Wq", (C, C), f32, kind="ExternalInput")
    bq = nc.dram_tensor("bq", (C,), f32, kind="ExternalInput")
    Wk = nc.dram_tensor("Wk", (C, C), f32, kind="ExternalInput")
    bk = nc.dram_tensor("bk", (C,), f32, kind="ExternalInput")
    Wv = nc.dram_tensor("Wv", (C, C), f32, kind="ExternalInput")
    bv = nc.dram_tensor("bv", (C,), f32, kind="ExternalInput")
    ln1_g = nc.dram_tensor("ln1_g", (C,), f32, kind="ExternalInput")
    ln1_b = nc.dram_tensor("ln1_b", (C,), f32, kind="ExternalInput")
    W1 = nc.dram_tensor("W1", (C, D), f32, kind="ExternalInput")
    b1 = nc.dram_tensor("b1", (D,), f32, kind="ExternalInput")
    W2 = nc.dram_tensor("W2", (D, C), f32, kind="ExternalInput")
    b2 = nc.dram_tensor("b2", (C,), f32, kind="ExternalInput")
    ln2_g = nc.dram_tensor("ln2_g", (C,), f32, kind="ExternalInput")
    ln2_b = nc.dram_tensor("ln2_b", (C,), f32, kind="ExternalInput")
    out = nc.dram_tensor("out", (M, C), f32, kind="ExternalOutput")

    def bcast(handle):
        """AP replicating a [C]-shaped dram vector across all partitions."""
        ap = handle[:]
        return bass.AP(tensor=ap.tensor, offset=ap.offset, ap=[[0, P], *ap.ap])

    def r(ap):
        return ap.bitcast(f32r)

    with tile.TileContext(nc) as tc, ExitStack() as top:
        const = top.enter_context(tc.tile_pool(name="const", bufs=1))
        dram = top.enter_context(tc.tile_pool(name="dram", bufs=1, space="DRAM"))

        # ---- constants -------------------------------------------------
        ident = const.tile([P, P], f32)
        make_identity(nc, ident)
        eps_t = const.tile([P, 1], f32)
        nc.vector.memset(eps_t, EPS)
        bq_t = const.tile([P, CK], f32)
        nc.sync.dma_start(bq_t, bq[:].rearrange("(c p) -> p c", p=P))
        bk_t = const.tile([P, CK], f32)
        nc.sync.dma_start(bk_t, bk[:].rearrange("(c p) -> p c", p=P))
        b1_t = const.tile([P, DC], f32)
        nc.sync.dma_start(b1_t, b1[:].rearrange("(c p) -> p c", p=P))
        bv_bc = const.tile([P, C], f32)
        nc.gpsimd.dma_start(bv_bc, bcast(bv))
        b2_bc = const.tile([P, C], f32)
        nc.gpsimd.dma_start(b2_bc, bcast(b2))
        g1_bc = const.tile([P, C], f32)
        nc.gpsimd.dma_start(g1_bc, bcast(ln1_g))
        b1ln_bc = const.tile([P, C], f32)
        nc.gpsimd.dma_start(b1ln_bc, bcast(ln1_b))
        g2_bc = const.tile([P, C], f32)
        nc.gpsimd.dma_start(g2_bc, bcast(ln2_g))
        b2ln_bc = const.tile([P, C], f32)
        nc.gpsimd.dma_start(b2ln_bc, bcast(ln2_b))

        # attention tail (query tiles 4..7) spilled to DRAM as raw exp()
        tail_dram = dram.tile([NC, P, C], bf16)

        # h lives to the very end
        h_pool = top.enter_context(tc.tile_pool(name="hp", bufs=1))
        h_sb = h_pool.tile([P, MT, C], f32)

        s1 = top.enter_context(ExitStack())
        # long-lived pools must open before the attention-scoped ones (LIFO)
        sC = s1.enter_context(tc.tile_pool(name="sC", bufs=1))       # pat_tm
        vpool = s1.enter_context(tc.tile_pool(name="vp", bufs=NC))   # v chunks
        x_ps_pool = s1.enter_context(tc.tile_pool(name="x_ps", bufs=4, space="PSUM"))

        with ExitStack() as mid:
            # pools that die when the attention stream finishes
            sA = mid.enter_context(tc.tile_pool(name="sA", bufs=1))       # patT
            sB = mid.enter_context(tc.tile_pool(name="sB", bufs=1))       # weights qkv, qT
            io = mid.enter_context(tc.tile_pool(name="io", bufs=3))       # img loads
            kvp = mid.enter_context(tc.tile_pool(name="kvp", bufs=2))     # kvT_t, kT_t
            att = mid.enter_context(tc.tile_pool(name="att", bufs=3))     # attn chunks
            sm = mid.enter_context(tc.tile_pool(name="sm", bufs=4))       # softmax scalars
            wstp = mid.enter_context(tc.tile_pool(name="wstp", bufs=1))   # weight staging
            # psum
            tp_ps = mid.enter_context(tc.tile_pool(name="tp_ps", bufs=1, space="PSUM"))
            mm_ps = mid.enter_context(tc.tile_pool(name="mm_ps", bufs=1, space="PSUM"))
            sc_ps = mid.enter_context(tc.tile_pool(name="sc_ps", bufs=1, space="PSUM"))

            # ---- load pattern (token-major) and build patT ------------
            pat_tm = sC.tile([P, MT, C], f32)
            nc.sync.dma_start(pat_tm, pattern[:, :].rearrange("(t p) c -> p t c", p=P))
            patT = sA.tile([P, CK, M], f32r)
            for mt in range(MT):
                for ci in range(CK):
                    tp = tp_ps.tile([P, P], f32, tag="tp")
                    nc.tensor.transpose(tp, pat_tm[:, mt, ci * P:(ci + 1) * P], ident)
                    nc.vector.tensor_copy(patT[:, ci, mt * P:(mt + 1) * P], tp)

            # ---- weights (DMA to f32 staging, round-copy to f32r) ------
            Wq_sb = sB.tile([P, CK, C], f32r)
            Wk_sb = sB.tile([P, CK, C], f32r)
            Wv_sb = sB.tile([P, CK, C], f32r)
            for dst, src in ((Wq_sb, Wq), (Wk_sb, Wk), (Wv_sb, Wv)):
                wst = wstp.tile([P, CK, C], f32, tag="wst")
                nc.sync.dma_start(wst, src[:, :].rearrange("(c p) n -> p c n", p=P))
                nc.vector.tensor_copy(dst, wst)

            # ---- qT = (pattern @ Wq + bq)^T  [C-part, M] ---------------
            qT = sB.tile([P, CK, M], f32r)
            for co in range(CK):
                for mh in range(2):
                    ps = mm_ps.tile([P, 512], f32, tag="mm")
                    for ci in range(CK):
                        nc.tensor.matmul(
                            ps, r(Wq_sb[:, ci, co * P:(co + 1) * P]),
                            r(patT[:, ci, mh * 512:(mh + 1) * 512]),
                            start=(ci == 0), stop=(ci == CK - 1))
                    nc.scalar.activation(
                        qT[:, co, mh * 512:(mh + 1) * 512], ps, Act.Identity,
                        bias=bq_t[:, co:co + 1])

            # ---- x psum accumulators (query tiles 0..3) ----------------
            x_ps = [x_ps_pool.tile([P, C], f32, tag="x", name=f"xps{i}") for i in range(4)]

            v_tiles = []
            # ================= fused projection + attention =============
            for t in range(NT):
                # kv^T for this 512-token tile
                kvT_t = kvp.tile([P, CK, 512], f32r, tag="kvT")
                for ncc in range(4):
                    row = (t * 4 + ncc) * P
                    ift = io.tile([P, C], f32, tag="if")
                    nc.sync.dma_start(ift, img_feat[row:row + P, :])
                    ipt = io.tile([P, C], f32, tag="ip")
                    nc.sync.dma_start(ipt, img_pos[row:row + P, :])
                    kvt = io.tile([P, C], f32, tag="kv")
                    nc.gpsimd.tensor_add(kvt, ift, ipt)
                    for ci in range(CK):
                        tp = tp_ps.tile([P, P], f32, tag="tp")
                        nc.tensor.transpose(tp, kvt[:, ci * P:(ci + 1) * P], ident)
                        nc.vector.tensor_copy(
                            kvT_t[:, ci, ncc * P:(ncc + 1) * P], tp)

                # k^T tile [C-part, 512] with bias
                kT_t = kvp.tile([P, CK, 512], f32r, tag="kT")
                for co in range(CK):
                    ps = mm_ps.tile([P, 512], f32, tag="mm")
                    for ci in range(CK):
                        nc.tensor.matmul(
                            ps, r(Wk_sb[:, ci, co * P:(co + 1) * P]),
                            r(kvT_t[:, ci, :]),
                            start=(ci == 0), stop=(ci == CK - 1))
                    nc.scalar.activation(
                        kT_t[:, co, :], ps, Act.Identity, bias=bk_t[:, co:co + 1])

                # v chunks [token-part, C] bf16 with bias
                for ncc in range(4):
                    ps = mm_ps.tile([P, 512], f32, tag="mm")
                    for ci in range(CK):
                        nc.tensor.matmul(
                            ps, r(kvT_t[:, ci, ncc * P:(ncc + 1) * P]),
                            r(Wv_sb[:, ci, :]),
                            start=(ci == 0), stop=(ci == CK - 1))
                    vt = vpool.tile([P, C], bf16, tag="v")
                    nc.vector.tensor_tensor(vt, ps, bv_bc, op=Alu.add)
                    v_tiles.append(vt)

                # attention for the 4 chunks of this tile
                for ncc in range(4):
                    j = t * 4 + ncc
                    ps_s = sc_ps.tile([P, M], f32, tag="sc")
                    for mh in range(2):
                        for ci in range(CK):
                            nc.tensor.matmul(
                                ps_s[:, mh * 512:(mh + 1) * 512],
                                r(kT_t[:, ci, ncc * P:(ncc + 1) * P]),
                                r(qT[:, ci, mh * 512:(mh + 1) * 512]),
                                start=(ci == 0), stop=(ci == CK - 1))
                    negmax = sm.tile([P, 1], f32, tag="nm")
                    nc.vector.tensor_reduce(
                        negmax, ps_s[:, :], axis=AX.X, op=Alu.max, negate=True)
                    sums = sm.tile([P, 2], f32, tag="sums")
                    attn0 = att.tile([P, 512], bf16, tag="a0")
                    nc.scalar.activation(attn0, ps_s[:, :512], Act.Exp,
                                         bias=negmax, accum_out=sums[:, 0:1])
                    attn1 = att.tile([P, 512], bf16, tag="a1")
                    nc.scalar.activation(attn1, ps_s[:, 512:], Act.Exp,
                                         bias=negmax, accum_out=sums[:, 1:2])
                    nc.sync.dma_start(tail_dram[j], attn1)
                    denom = sm.tile([P, 1], f32, tag="dn")
                    nc.vector.tensor_reduce(denom, sums, axis=AX.X, op=Alu.add)
                    rcp = sm.tile([P, 1], f32, tag="rcp")
                    nc.vector.reciprocal(rcp, denom)
                    # fold softmax normalization into v
                    nc.vector.tensor_scalar_mul(v_tiles[j], v_tiles[j], rcp)
                    for ms in range(4):
                        nc.tensor.matmul(
                            x_ps[ms], attn0[:, ms * P:(ms + 1) * P], v_tiles[j],
                            start=(j == 0), stop=(j == NC - 1))

        # ---- pools for LN1 / attention post-pass (reuse mid space) -----
        ln1p = s1.enter_context(tc.tile_pool(name="ln1", bufs=3))
        tail_rd = s1.enter_context(tc.tile_pool(name="tr", bufs=4))

        def ln_apply(pool, src_psum, resid, g_bc, bln_bc, extra_bc, dst):
            """dst = LN(src + resid [+ extra]) * g + b   (dst/resid in SBUF)"""
            tpre = pool.tile([P, C], f32, tag="tpre")
            nc.vector.tensor_tensor(tpre, src_psum, resid, op=Alu.add)
            if extra_bc is not None:
                nc.gpsimd.tensor_add(tpre, tpre, extra_bc)
            stats = pool.tile([P, 6], f32, tag="st")
            nc.vector.bn_stats(stats, tpre)
            mv = pool.tile([P, 2], f32, tag="mv")
            nc.vector.bn_aggr(mv, stats)
            sd = pool.tile([P, 1], f32, tag="sd")
            nc.scalar.activation(sd, mv[:, 1:2], Act.Sqrt, bias=eps_t)
            rstd = pool.tile([P, 1], f32, tag="rs")
            nc.vector.reciprocal(rstd, sd)
            xc = pool.tile([P, C], f32, tag="xc")
            nc.vector.tensor_scalar(xc, tpre, mv[:, 0:1], rstd,
                                    op0=Alu.subtract, op1=Alu.mult)
            nc.gpsimd.tensor_mul(xc, xc, g_bc)
            nc.gpsimd.tensor_add(dst, xc, bln_bc)

        # LN1 for in-loop query tiles 0..3
        for ms in range(4):
            ln_apply(ln1p, x_ps[ms], pat_tm[:, ms, :], g1_bc, b1ln_bc, None,
                     h_sb[:, ms, :])

        # ---- attention post-pass: query tiles 4..7 from DRAM tail ------
        x_ps2 = [x_ps_pool.tile([P, C], f32, tag="x", name=f"xps2_{i}")
                 for i in range(4)]
        for j in range(NC):
            at = tail_rd.tile([P, C], bf16, tag="at")
            nc.sync.dma_start(at, tail_dram[j])
            for ms in range(4):
                nc.tensor.matmul(
                    x_ps2[ms], at[:, ms * P:(ms + 1) * P], v_tiles[j],
                    start=(j == 0), stop=(j == NC - 1))
        for ms in range(4):
            ln_apply(ln1p, x_ps2[ms], pat_tm[:, 4 + ms, :], g1_bc, b1ln_bc, None,
                     h_sb[:, 4 + ms, :])

        # ---- close attention-phase pools, open FFN pools ---------------
        s1.close()
        wpool = top.enter_context(tc.tile_pool(name="wp", bufs=1))
        wload = top.enter_context(tc.tile_pool(name="wl", bufs=2))
        hq = top.enter_context(tc.tile_pool(name="hq", bufs=1))
        ln2p = top.enter_context(tc.tile_pool(name="ln2", bufs=3))
        outp = top.enter_context(tc.tile_pool(name="outp", bufs=3))
        htp_ps = top.enter_context(tc.tile_pool(name="htp", bufs=2, space="PSUM"))
        y1_ps = top.enter_context(tc.tile_pool(name="y1ps", bufs=2, space="PSUM"))
        x2_ps = top.enter_context(tc.tile_pool(name="x2ps", bufs=2, space="PSUM"))

        # ---- FFN weights ----------------------------------------------
        W1_sb = wpool.tile([P, CK, D], bf16)
        for ci in range(CK):
            wt = wload.tile([P, D], f32, tag="w1l")
            nc.sync.dma_start(wt, W1[:, :].rearrange("(c p) d -> p c d", p=P)[:, ci, :])
            nc.gpsimd.tensor_copy(W1_sb[:, ci, :], wt)
        W2_sb = wpool.tile([P, DC, C], bf16)
        for dc4 in range(4):
            wt = wload.tile([P, 4, C], f32, tag="w2l")
            nc.sync.dma_start(
                wt, W2[:, :].rearrange("(c p) n -> p c n", p=P)[:, dc4 * 4:(dc4 + 1) * 4, :])
            nc.gpsimd.tensor_copy(W2_sb[:, dc4 * 4:(dc4 + 1) * 4, :], wt)

        # ---- h^T (bf16) ------------------------------------------------
        hT = hq.tile([P, CK, M], bf16)
        for mt in range(MT):
            for ci in range(CK):
                tp = htp_ps.tile([P, P], f32, tag="htp")
                nc.tensor.transpose(tp, h_sb[:, mt, ci * P:(ci + 1) * P], ident)
                nc.vector.tensor_copy(hT[:, ci, mt * P:(mt + 1) * P], tp)

        # ---- y1T = relu(W1^T @ h^T + b1)  [D-part, M] bf16 -------------
        y1T = hq.tile([P, DC, M], bf16)
        for dc in range(DC):
            for mh in range(2):
                ps = y1_ps.tile([P, 512], f32, tag="y1")
                for ci in range(CK):
                    nc.tensor.matmul(
                        ps, W1_sb[:, ci, dc * P:(dc + 1) * P],
                        hT[:, ci, mh * 512:(mh + 1) * 512],
                        start=(ci == 0), stop=(ci == CK - 1))
                nc.scalar.activation(
                    y1T[:, dc, mh * 512:(mh + 1) * 512], ps, Act.Relu,
                    bias=b1_t[:, dc:dc + 1])

        # ---- x2 = y1 @ W2 ; out = LN(h + x2 + b2) ----------------------
        for mt in range(MT):
            ps = x2_ps.tile([P, 512], f32, tag="x2")
            for dc in range(DC):
                nc.tensor.matmul(
                    ps, y1T[:, dc, mt * P:(mt + 1) * P], W2_sb[:, dc, :],
                    start=(dc == 0), stop=(dc == DC - 1))
            ot = outp.tile([P, C], f32, tag="ot")
            ln_apply(ln2p, ps, h_sb[:, mt, :], g2_bc, b2ln_bc, b2_bc, ot)
            nc.sync.dma_start(out[mt * P:(mt + 1) * P, :], ot)

    nc.finalize()
    return nc


def _get_nc():
    if "nc" not in _CACHE:
        _CACHE["nc"] = _build()
    return _CACHE["nc"]


def kernel(**inputs):
    from concourse import bass_utils

    nc = _get_nc()
    full = {k: np.ascontiguousarray(np.asarray(v, dtype=np.float32))
            for k, v in inputs.items()}
    in_maps = []
    for i in range(B):
        m = {
            "img_feat": full["img_feat"][i],
            "img_pos": full["img_pos"][i],
            "pattern_feat": full["pattern_feat"][i],
        }
        for w in ("Wq", "bq", "Wk", "bk", "Wv", "bv", "ln1_g", "ln1_b",
                  "W1", "b1", "W2", "b2", "ln2_g", "ln2_b"):
            m[w] = full[w]
        in_maps.append(m)
    res = bass_utils.run_bass_kernel_spmd(nc, in_maps, core_ids=list(range(B)))
    return np.stack([res.results[i]["out"] for i in range(B)], axis=0)
